# revision 50
# baseline (speedup 1.0000x reference)
"""Trainium2 Bass kernel for the GatedCRF 3D semseg loss.

Reformulation (validated vs reference to ~6e-7 rel in fp64):
With C=2 softmax channels, y0+y1=1. Let a = 1-2*y0, then per voxel-pair
  y0A*y1B + y1A*y0B = (1 - aA*aB)/2
so with E(l,delta) = exp(-0.5*((I[l+d]-I[l])/SIMG)^2 - 0.5*msq(delta)):
  loss*denom = sum_{d in HALF} [ sum_l E  -  sum_l E*aA*aB ] + G_total
where HALF is the 73 lexicographically-positive offsets of the 7x7x3
window and G_total is the out-of-bounds kernel mass
sum_l noob(l)*exp(-0.5*msq_c(l) - 0.5*(I_l/SIMG)^2).

Validity masking is data-driven: out-of-volume halo voxels carry J = BIG
so any one-sided-OOB pair gets E = exp(-huge) = 0, and both-OOB pairs
have aA = aB = 1 (u=0 pad) so E - E*aA*aB cancels exactly.

sum_l E rides free on the Exp's accum_out. The product side factors as
sum_l aA * Q(l) with Q = sum_d E_d * aB_d accumulated across slots by
in-place DVE adds (first pair's v-mult writes Q directly), so each
offset needs only {sub, Square, Exp, v-mult, Q+=v} and the product
reduction is ONE final STT-with-accum.

Engine notes (measured on HW):
 - GPSIMD shares its SBUF port with the DVE; running it concurrently
   slows DVE 3.5x -> everything stays on DVE+ACT.
 - ACT is 1x-rate ((FD+352)/1.2GHz) regardless of dtype.
 - DVE fp16 TT with step-1 4B-aligned operands runs 2x
   ((FD/2+151)/0.96GHz); STT runs 1x. In-place accumulate TTs and
   3-free-dim register APs with stride-0 broadcast all run at full 2x.
 - The input DMA fabric sustains only ~250GB/s total (shared across
   the SP and ACT HW DGE rings), so input bytes are minimized: J and
   a ship as SINGLE fp16 halo-packed arrays (852KB each, J on the SP
   ring, a on the ACT ring, meta first) and the compact 4B-aligned
   central tiles Jc/Ac ([128, 1088]) are built on-chip with two DVE
   copies instead of shipping shifted duplicates.

dd=+-1 offset pairs share one bias (dd^2) and are fused per-slot via a
stride-2 AP dim (even base -> 2x mode) with a stride-0 broadcast
central side reading Jc/Ac. dd=0 singles have odd bases and run at 1x;
their Exp/Square are unaffected (ACT is 1x anyway).
Layout: partition p = 16*h_blk + w_blk is a (4h x 8w) block with
halos; only dh >= 0 is ever read (top h-halo trimmed: 7 stored rows).

Per-core slots (SPMD; offsets/biases are per-core data):
3 dd=+-1 pair slots (6 offsets) + 3 dd=0 single slots + 1/8 of the
lone (0,0,1) offset (spatially split mini-slot) + 1/8 of the G-pass.
"""

import numpy as np

# problem constants (hardcoded per contract)
H, W, D = 64, 64, 32
SXY, SIMG = 5.0, 0.1
RH, RW, RD = 3, 3, 1
NCORES = 8
NPAIR, NSING = 3, 3
BH, BW = 4, 8                    # central block per partition
NHB, NWB = H // BH, W // BW      # 16 x 8 blocks = 128 partitions
SH = BH + RH                     # 7 stored h rows (top halo trimmed)
SW = BW + 2 * RW                 # 14
SD = D + 2 * RD                  # 34
FREE = SH * SW * SD              # 3332 stored elems per partition
ROW = BW * SD                    # 272: fused (w,d) run per h row
PAD = 8
JP = FREE + PAD                  # single-copy row length (3340, even)
CENT = RW * SD + RD              # 103 (odd) central base in copy-0
MROW = BH * ROW // NCORES        # 136: mini/G slice length per core
NQ = BH * ROW                    # 1088
NCOLS = 12                       # E sums(7) | spare | Sfin | Smini | G | S2
NMETA = 136 + 7 + 17             # t3 slice | biases | int offsets
SQ = float(np.sqrt(0.5) / SIMG)  # sqrt(50)
S2 = float(0.5 / SIMG ** 2)      # 50
BIG = 28.0                       # halo marker: max |d|=BIG+6 -> q<=57800
NEG = -1.0e4
DENOM = float(H * W * D)
OFFB = RH * SW * SD + 2 * RW * SD + 2 * RD   # 1634: max slot base


def _pair_single_slots():
    """pairs/singles = (dh,dw) lists; pairs fuse dd=+-1, singles dd=0."""
    pairs, singles = [], []
    for dh in range(0, RH + 1):
        for dw in range(-RW, RW + 1):
            if (dh > 0) or (dh == 0 and dw > 0):
                pairs.append((dh, dw))
                singles.append((dh, dw))
    assert len(pairs) == 24 and len(singles) == 24
    return pairs, singles


def _pack(v, pad_val):
    """(H, W, D) -> [128, FREE]: per-partition block + trimmed halos."""
    vp = np.pad(v.astype(np.float32), ((RH, RH), (RW, RW), (RD, RD)),
                constant_values=pad_val)
    out = np.empty((128, SH, SW, SD), np.float32)
    for hb in range(NHB):
        for wb in range(NWB):
            out[hb * NWB + wb] = vp[hb * BH + RH:hb * BH + RH + SH,
                                    wb * BW:wb * BW + SW, :]
    return out.reshape(128, FREE)


def _single_f16(flat, pad_val):
    """[128, FREE] -> fp16 [128, JP] with pad columns."""
    out = np.full((128, JP), pad_val, np.float16)
    out[:, :FREE] = flat.astype(np.float16)
    return out


def _build_nc():
    import concourse.bass as bass
    import concourse.bacc as bacc
    import concourse.mybir as mybir
    from concourse.tile import TileContext

    f32, f16, i32 = mybir.dt.float32, mybir.dt.float16, mybir.dt.int32
    AF = mybir.ActivationFunctionType
    OP = mybir.AluOpType
    ET = mybir.EngineType

    nc = bacc.Bacc("TRN2", target_bir_lowering=False, debug=False)
    jad = nc.dram_tensor("jad", [128, 2 * JP], f16, kind="ExternalInput")
    meta = nc.dram_tensor("meta", [128, NMETA], f32, kind="ExternalInput")
    out = nc.dram_tensor("out", [128, NCOLS], f32, kind="ExternalOutput")

    # patterns: [partition][(pair)][h][flat (w,d) row]
    JPP = 2 * JP                  # fused [J | A] row length (13.4KB rows)
    P1 = [[JPP, 128], [SW * SD, BH], [1, ROW]]                # single B-view
    P2 = [[JPP, 128], [2, 2], [SW * SD, BH], [1, ROW]]        # dd=+-1 pair
    PM = [[JPP, 128], [1, MROW]]                              # mini B slice
    # central views live in the compact on-chip Jc/Ac tiles [128, NQ]
    C1 = [[NQ, 128], [ROW, BH], [1, ROW]]                     # central
    C2B = [[NQ, 128], [0, 2], [ROW, BH], [1, ROW]]            # bcast pair
    CM = [[NQ, 128], [1, MROW]]                               # mini/G slice

    with TileContext(nc) as tc:
        with tc.tile_pool(name="pers", bufs=1) as pers, \
             tc.tile_pool(name="dp2", bufs=3) as dp2, \
             tc.tile_pool(name="qp2", bufs=3) as qp2, \
             tc.tile_pool(name="ep2", bufs=3) as ep2, \
             tc.tile_pool(name="vp2", bufs=3) as vp2, \
             tc.tile_pool(name="dp1", bufs=3) as dp1, \
             tc.tile_pool(name="qp1", bufs=3) as qp1, \
             tc.tile_pool(name="ep1", bufs=3) as ep1, \
             tc.tile_pool(name="vp1", bufs=3) as vp1, \
             tc.tile_pool(name="gp", bufs=1) as gp:
            JA = pers.tile([128, JPP], f16, tag="JA")
            Jc = pers.tile([128, NQ], f16, tag="Jc")
            metatile = pers.tile([128, NMETA], f32, tag="meta")
            acc = pers.tile([128, NCOLS], f32, tag="acc")
            Q1 = pers.tile([128, NQ], f16, tag="Q1")
            Q2 = pers.tile([128, 2 * NQ], f16, tag="Q2")
            fin = pers.tile([128, NQ], f16, tag="fin")

            nc.vector.memset(acc[:], 0.0)
            # meta first on the SP ring (tiny; the register loads gate all
            # compute), then J on SP; A rides the ACT ring in parallel.
            nc.sync.dma_start(metatile[:], meta[:])
            nc.sync.dma_start(JA[:], jad[:])

            t3v = metatile[:, 0:MROW]
            BIAS0 = MROW
            biasv = metatile[:, BIAS0:BIAS0 + 7]   # pair0..2, sing0..2, mini
            INT0 = BIAS0 + 7
            # ints: pair joffs 0..2 | single joffs 3..5 | mini jB 6 |
            #       mini cA 7 | gjoff 8
            _, dvv = nc.values_load_multi_w_load_instructions(
                metatile[0:1, INT0:INT0 + 16].bitcast(i32),
                engines=(ET.DVE,), min_val=0,
                max_val=JP + CENT + 1 + OFFB,
                skip_runtime_bounds_check=True)
            pv, sv, mv = dvv[0:3], dvv[3:6], dvv[6:8]
            pva, sva, mba, mca = dvv[8:11], dvv[11:14], dvv[14], dvv[15]
            gval = nc.values_load(
                metatile[0:1, INT0 + 16:INT0 + 17].bitcast(i32),
                engines=(ET.Activation,), min_val=0, max_val=NQ - MROW,
                skip_runtime_bounds_check=True)

            # compact central copies (strided -> contiguous, on DVE)
            Jsrc = bass.AP(JA.tensor, CENT, P1)
            nc.vector.tensor_copy(
                Jc[:].rearrange("p (a b) -> p a b", a=BH, b=ROW), Jsrc)

            J_A1 = bass.AP(Jc.tensor, 0, C1)
            J_A2 = bass.AP(Jc.tensor, 0, C2B)
            a_A1 = bass.AP(JA.tensor, JP + CENT, P1)

            # ---- mini + G first: they only need J/meta, fill the DMA
            # shadow on both engines (v/ms parts emitted later) ----
            md = gp.tile([128, MROW], f16, tag="md")
            nc.vector.tensor_tensor(
                md[:], bass.AP(JA.tensor, mv[0], PM),
                bass.AP(Jc.tensor, mv[1], CM), OP.subtract)
            mq = gp.tile([128, MROW], f16, tag="mq")
            nc.vector.tensor_tensor(mq[:], md[:], md[:], OP.mult)
            me = gp.tile([128, MROW], f16, tag="me")
            nc.scalar.activation(me[:], mq[:], AF.Exp, scale=-S2,
                                 bias=biasv[:, 6:7],
                                 accum_out=acc[:, 6:7])
            qg = gp.tile([128, MROW], f16, tag="qg")
            nc.scalar.activation(
                qg[:], bass.AP(Jc.tensor, gval, CM), AF.Square, scale=SQ)
            ag = gp.tile([128, MROW], f32, tag="ag")
            nc.vector.scalar_tensor_tensor(
                ag[:], qg[:], -1.0, t3v[:, 0:MROW], OP.mult, OP.add)
            eg = gp.tile([128, MROW], f16, tag="eg")
            nc.scalar.activation(eg[:], ag[:], AF.Exp,
                                 accum_out=acc[:, 10:11])

            # front-load every sub on the DVE queue so the ACT sq/exp
            # stream ramps immediately and the later v-chain never waits
            def pair_front(j):
                dt = dp2.tile([128, 2, BH, ROW], f16, tag="d2")
                nc.vector.tensor_tensor(
                    dt[:], bass.AP(JA.tensor, pv[j], P2), J_A2, OP.subtract)
                qt = qp2.tile([128, 2, BH, ROW], f16, tag="q2")
                nc.scalar.activation(qt[:], dt[:], AF.Square, scale=SQ)
                et = ep2.tile([128, 2, BH, ROW], f16, tag="e2")
                nc.scalar.activation(et[:], qt[:], AF.Exp, scale=-1.0,
                                     bias=biasv[:, j:j + 1],
                                     accum_out=acc[:, j:j + 1])
                return et

            def pair_back(j, et, first):
                if first:
                    nc.vector.tensor_tensor(
                        Q2[:].rearrange("p (a b c) -> p a b c", a=2, b=BH),
                        et[:], bass.AP(JA.tensor, pva[j], P2), OP.mult)
                else:
                    vt = vp2.tile([128, 2, BH, ROW], f16, tag="v2")
                    nc.vector.tensor_tensor(
                        vt[:], et[:], bass.AP(JA.tensor, pva[j], P2), OP.mult)
                    nc.vector.tensor_tensor(
                        Q2[:], Q2[:],
                        vt[:].rearrange("p a b c -> p (a b c)"), OP.add)

            def single_front(j):
                dt = dp1.tile([128, BH, ROW], f16, tag="d1")
                nc.vector.tensor_tensor(
                    dt[:], bass.AP(JA.tensor, sv[j], P1), J_A1, OP.subtract)
                qt = qp1.tile([128, BH, ROW], f16, tag="q1")
                nc.scalar.activation(qt[:], dt[:], AF.Square, scale=SQ)
                et = ep1.tile([128, BH, ROW], f16, tag="e1")
                nc.scalar.activation(et[:], qt[:], AF.Exp, scale=-1.0,
                                     bias=biasv[:, NPAIR + j:NPAIR + j + 1],
                                     accum_out=acc[:, NPAIR + j:NPAIR + j + 1])
                return et

            def single_back(j, et, mode):
                if mode == "init":
                    nc.vector.tensor_tensor(
                        Q1[:].rearrange("p (a b) -> p a b", a=BH),
                        et[:], bass.AP(JA.tensor, sva[j], P1), OP.mult)
                    return
                vt = vp1.tile([128, BH, ROW], f16, tag="v1")
                nc.vector.tensor_tensor(
                    vt[:], et[:], bass.AP(JA.tensor, sva[j], P1), OP.mult)
                if mode == "add":
                    nc.vector.tensor_tensor(
                        Q1[:], Q1[:],
                        vt[:].rearrange("p a b -> p (a b)"), OP.add)
                else:
                    st = gp.tile([128, BH, ROW], f16, tag="st")
                    nc.vector.scalar_tensor_tensor(
                        st[:], vt[:], 1.0, a_A1, OP.mult, OP.mult,
                        accum_out=acc[:, 11:12])

            e_s0 = single_front(0)
            e_p0 = pair_front(0)
            e_p1 = pair_front(1)
            e_s1 = single_front(1)
            e_p2 = pair_front(2)
            single_back(0, e_s0, "init")
            pair_back(0, e_p0, True)
            pair_back(1, e_p1, False)
            single_back(1, e_s1, "add")
            pair_back(2, e_p2, False)

            # single2 front half (sub/Square/Exp) - independent of Q1/Q2
            s2d = dp1.tile([128, BH, ROW], f16, tag="d1")
            nc.vector.tensor_tensor(
                s2d[:], bass.AP(JA.tensor, sv[2], P1), J_A1, OP.subtract)
            s2q = qp1.tile([128, BH, ROW], f16, tag="q1")
            nc.scalar.activation(s2q[:], s2d[:], AF.Square, scale=SQ)
            s2e = ep1.tile([128, BH, ROW], f16, tag="e1")
            nc.scalar.activation(s2e[:], s2q[:], AF.Exp, scale=-1.0,
                                 bias=biasv[:, NPAIR + 2:NPAIR + 3],
                                 accum_out=acc[:, NPAIR + 2:NPAIR + 3])

            # mini product part (needs A)
            mvt = gp.tile([128, MROW], f16, tag="mv")
            nc.vector.tensor_tensor(
                mvt[:], me[:], bass.AP(JA.tensor, mba, PM), OP.mult)
            ms = gp.tile([128, MROW], f16, tag="ms")
            nc.vector.scalar_tensor_tensor(
                ms[:], mvt[:], 1.0, bass.AP(JA.tensor, mca, PM),
                OP.mult, OP.mult, accum_out=acc[:, 9:10])

            # collapse pair lanes, fold into Q1, and run the final
            # reduction BEFORE single2's product ops so merge+fin overlap
            # single2's Square/Exp on ACT (single2 bypasses Q1 via its
            # own STT column, so the order is sound)
            nc.vector.tensor_tensor(Q2[:, 0:NQ], Q2[:, 0:NQ],
                                    Q2[:, NQ:2 * NQ], OP.add)
            nc.vector.tensor_tensor(Q1[:], Q1[:], Q2[:, 0:NQ], OP.add)
            # ---- final: col8 = sum aA * Q1 ----
            nc.vector.scalar_tensor_tensor(
                fin[:].rearrange("p (a b) -> p a b", a=BH, b=ROW),
                Q1[:].rearrange("p (a b) -> p a b", a=BH, b=ROW),
                1.0, a_A1, OP.mult, OP.mult, accum_out=acc[:, 8:9])

            # single2 back half: v-mult + STT into its own column
            s2v = vp1.tile([128, BH, ROW], f16, tag="v1")
            nc.vector.tensor_tensor(
                s2v[:], s2e[:], bass.AP(JA.tensor, sva[2], P1), OP.mult)
            s2s = gp.tile([128, BH, ROW], f16, tag="st")
            nc.vector.scalar_tensor_tensor(
                s2s[:], s2v[:], 1.0, a_A1, OP.mult, OP.mult,
                accum_out=acc[:, 11:12])

            nc.sync.dma_start(out[:], acc[:])
    nc.compile()
    return nc


def _host_tables(sample, spacing):
    """Per-core meta arrays."""
    sp = np.asarray(spacing, dtype=np.float64)[:, 0]
    pairs, singles = _pair_single_slots()

    # t3 = ln(noob) - 0.5*msq_center (NEG where noob == 0), central packing
    h = np.arange(H)[:, None, None]
    w = np.arange(W)[None, :, None]
    d = np.arange(D)[None, None, :]
    msq_c = ((sp[0] * h) ** 2 + (sp[1] * w) ** 2 + (sp[2] * d) ** 2) / SXY ** 2
    cnt = ((np.minimum(h, RH) + np.minimum(H - 1 - h, RH) + 1)
           * (np.minimum(w, RW) + np.minimum(W - 1 - w, RW) + 1)
           * (np.minimum(d, RD) + np.minimum(D - 1 - d, RD) + 1))
    noob = (2 * RH + 1) * (2 * RW + 1) * (2 * RD + 1) - cnt
    t3full = np.where(noob > 0, np.log(np.maximum(noob, 1)) - 0.5 * msq_c, NEG)
    t3b = np.empty((128, BH, BW, D), np.float32)
    for hb in range(NHB):
        for wb in range(NWB):
            t3b[hb * NWB + wb] = t3full[hb * BH:(hb + 1) * BH,
                                        wb * BW:(wb + 1) * BW, :]
    t3flat = t3b.reshape(128, BH * BW * D)

    def bias_of(dh, dw, dd):
        msq = ((sp[0] * dh) ** 2 + (sp[1] * dw) ** 2
               + (sp[2] * dd) ** 2) / SXY ** 2
        return -0.5 * msq

    metas = []
    for c in range(NCORES):
        m = np.zeros((128, NMETA), np.float32)
        # G/mini slice: h-row c//2, col half c%2 of the 272-wide flat row
        sl = (c // 2) * SW * SD + (c % 2) * MROW
        # map on-chip slice positions (d incl halo) to central t3 values
        t3s = np.full((128, MROW), NEG, np.float32)
        r0 = c // 2
        base = (c % 2) * MROW
        for i in range(MROW):
            wcol, dcol = divmod(base + i, SD)
            if 1 <= dcol <= D:
                t3s[:, i] = t3flat[:, (r0 * BW + wcol) * D + (dcol - 1)]
        m[:, 0:MROW] = t3s

        B0 = MROW
        joff_p = np.zeros(3, np.int32)
        joff_s = np.zeros(3, np.int32)
        for j in range(NPAIR):
            dh, dw = pairs[3 * c + j]
            j0 = dh * SW * SD + (RW + dw) * SD + (RD - 1)   # dd=-1 lane
            assert j0 % 2 == 0
            joff_p[j] = j0                                  # copy-0 coords
            m[:, B0 + j] = bias_of(dh, dw, 1)
        for j in range(NSING):
            dh, dw = singles[3 * c + j]
            j0 = dh * SW * SD + (RW + dw) * SD + RD         # dd=0
            assert j0 % 2 == 1
            joff_s[j] = j0                                  # copy-0, odd
            m[:, B0 + NPAIR + j] = bias_of(dh, dw, 0)
        m[:, B0 + 6] = bias_of(0, 0, 1)                     # mini
        I0 = B0 + 7
        m[0, I0:I0 + 3] = joff_p.view(np.float32)
        m[0, I0 + 3:I0 + 6] = joff_s.view(np.float32)
        # mini: B = central+1 (dd=+1) in stored J coords; A-side and the
        # G input are slices of the compact central Jc/Ac tiles; the
        # second block of offsets addresses the A half (+JP) of the
        # fused [J | A] tile
        mb = CENT + 1 + sl
        csl = (c // 2) * ROW + (c % 2) * MROW
        m[0, I0 + 6] = np.int32(mb).view(np.float32)
        m[0, I0 + 7] = np.int32(csl).view(np.float32)
        m[0, I0 + 8:I0 + 11] = (joff_p + JP).view(np.float32)
        m[0, I0 + 11:I0 + 14] = (joff_s + JP).view(np.float32)
        m[0, I0 + 14] = np.int32(mb + JP).view(np.float32)
        m[0, I0 + 15] = np.int32(JP + CENT + sl).view(np.float32)
        m[0, I0 + 16] = np.int32(csl).view(np.float32)      # gjoff
        metas.append(m)
    return metas


def _host_inputs(y_hat_softmax, sample, spacing):
    y0 = np.asarray(y_hat_softmax, dtype=np.float32)[0, 0]
    I = np.asarray(sample, dtype=np.float32)[0, 0]
    jd = _single_f16(_pack(I, BIG), 0.0)
    ad = _single_f16(_pack(1.0 - 2.0 * y0, 1.0), 1.0)
    jad = np.concatenate([jd, ad], axis=1)
    metas = _host_tables(sample, spacing)
    return [{"jad": jad, "meta": metas[c]} for c in range(NCORES)]


def kernel(y_hat_softmax, sample, spacing):
    from concourse.bass_utils import run_bass_kernel_spmd

    in_maps = _host_inputs(y_hat_softmax, sample, spacing)
    nc = _build_nc()
    res = run_bass_kernel_spmd(nc, in_maps, core_ids=list(range(NCORES)))
    total = 0.0
    for r in res.results:
        o = r["out"].astype(np.float64)
        # cols 0..6 = sum E per slot (pairs, singles, mini); col8 = final
        # sum aA*Q1; col9 = mini product sum; col10 = G; col11 = single2
        # product sum
        total += (o[:, 0:7].sum() - o[:, 8].sum() - o[:, 9].sum()
                  + o[:, 10].sum() - o[:, 11].sum())
    return np.array(total / DENOM, dtype=np.float32)


if __name__ == "__main__":
    rng = np.random.default_rng(0)
    logits = rng.standard_normal((1, 2, H, W, D)).astype(np.float32)
    e = np.exp(logits - logits.max(axis=1, keepdims=True))
    yh = (e / e.sum(axis=1, keepdims=True)).astype(np.float32)
    smp = rng.standard_normal((1, 1, H, W, D)).astype(np.float32)
    spc = rng.uniform(0.5, 2.0, (3, 1)).astype(np.float32)
    print(kernel(yh, smp, spc))


# revision 52
# speedup vs baseline: 1.0314x; 1.0314x over previous
"""Trainium2 Bass kernel for the GatedCRF 3D semseg loss.

Reformulation (validated vs reference to ~6e-7 rel in fp64):
With C=2 softmax channels, y0+y1=1. Let a = 1-2*y0, then per voxel-pair
  y0A*y1B + y1A*y0B = (1 - aA*aB)/2
so with E(l,delta) = exp(-0.5*((I[l+d]-I[l])/SIMG)^2 - 0.5*msq(delta)):
  loss*denom = sum_{d in HALF} [ sum_l E  -  sum_l E*aA*aB ] + G_total
where HALF is the 73 lexicographically-positive offsets of the 7x7x3
window and G_total is the out-of-bounds kernel mass
sum_l noob(l)*exp(-0.5*msq_c(l) - 0.5*(I_l/SIMG)^2).

Validity masking is data-driven: out-of-volume halo voxels carry J = BIG
so any one-sided-OOB pair gets E = exp(-huge) = 0, and both-OOB pairs
have aA = aB = 1 (u=0 pad) so E - E*aA*aB cancels exactly.

sum_l E rides free on the Exp's accum_out. The product side factors as
sum_l aA * Q(l) with Q = sum_d E_d * aB_d accumulated across slots by
in-place DVE adds (first pair's v-mult writes Q directly), so each
offset needs only {sub, Square, Exp, v-mult, Q+=v} and the product
reduction is ONE final STT-with-accum.

Engine notes (measured on HW):
 - GPSIMD shares its SBUF port with the DVE; running it concurrently
   slows DVE 3.5x -> everything stays on DVE+ACT.
 - ACT is 1x-rate ((FD+352)/1.2GHz) regardless of dtype.
 - DVE fp16 TT with step-1 4B-aligned operands runs 2x
   ((FD/2+151)/0.96GHz); STT runs 1x. In-place accumulate TTs and
   3-free-dim register APs with stride-0 broadcast all run at full 2x.
 - The input DMA fabric sustains only ~250GB/s total (shared across
   the SP and ACT HW DGE rings), so input bytes are minimized: J and
   a ship as SINGLE fp16 halo-packed arrays (852KB each, J on the SP
   ring, a on the ACT ring, meta first) and the compact 4B-aligned
   central tiles Jc/Ac ([128, 1088]) are built on-chip with two DVE
   copies instead of shipping shifted duplicates.

dd=+-1 offset pairs share one bias (dd^2) and are fused per-slot via a
stride-2 AP dim (even base -> 2x mode) with a stride-0 broadcast
central side reading Jc/Ac. dd=0 singles have odd bases and run at 1x;
their Exp/Square are unaffected (ACT is 1x anyway).
Layout: partition p = 16*h_blk + w_blk is a (4h x 8w) block with
halos; only dh >= 0 is ever read (top h-halo trimmed: 7 stored rows).

Per-core slots (SPMD; offsets/biases are per-core data):
3 dd=+-1 pair slots (6 offsets) + 3 dd=0 single slots + 1/8 of the
lone (0,0,1) offset (spatially split mini-slot) + 1/8 of the G-pass.
"""

import numpy as np

# problem constants (hardcoded per contract)
H, W, D = 64, 64, 32
SXY, SIMG = 5.0, 0.1
RH, RW, RD = 3, 3, 1
NCORES = 8
NPAIR, NSING = 3, 3
BH, BW = 4, 8                    # central block per partition
NHB, NWB = H // BH, W // BW      # 16 x 8 blocks = 128 partitions
SH = BH + RH                     # 7 stored h rows (top halo trimmed)
SW = BW + 2 * RW                 # 14
SD = D + 2 * RD                  # 34
FREE = SH * SW * SD              # 3332 stored elems per partition
ROW = BW * SD                    # 272: fused (w,d) run per h row
PAD = 8
JP = FREE + PAD                  # single-copy row length (3340, even)
CENT = RW * SD + RD              # 103 (odd) central base in copy-0
MROW = BH * ROW // NCORES        # 136: mini/G slice length per core
NQ = BH * ROW                    # 1088
NCOLS = 12                       # E sums(7) | spare | Sfin | Smini | G | S2
NMETA = 136 + 7 + 17             # t3 slice | biases | int offsets
SQ = float(np.sqrt(0.5) / SIMG)  # sqrt(50)
S2 = float(0.5 / SIMG ** 2)      # 50
BIG = 28.0                       # halo marker: max |d|=BIG+6 -> q<=57800
NEG = -1.0e4
DENOM = float(H * W * D)
OFFB = RH * SW * SD + 2 * RW * SD + 2 * RD   # 1634: max slot base


def _pair_single_slots():
    """pairs/singles = (dh,dw) lists; pairs fuse dd=+-1, singles dd=0."""
    pairs, singles = [], []
    for dh in range(0, RH + 1):
        for dw in range(-RW, RW + 1):
            if (dh > 0) or (dh == 0 and dw > 0):
                pairs.append((dh, dw))
                singles.append((dh, dw))
    assert len(pairs) == 24 and len(singles) == 24
    return pairs, singles


def _pack(v, pad_val):
    """(H, W, D) -> [128, FREE]: per-partition block + trimmed halos."""
    vp = np.pad(v.astype(np.float32), ((RH, RH), (RW, RW), (RD, RD)),
                constant_values=pad_val)
    out = np.empty((128, SH, SW, SD), np.float32)
    for hb in range(NHB):
        for wb in range(NWB):
            out[hb * NWB + wb] = vp[hb * BH + RH:hb * BH + RH + SH,
                                    wb * BW:wb * BW + SW, :]
    return out.reshape(128, FREE)


def _single_f16(flat, pad_val):
    """[128, FREE] -> fp16 [128, JP] with pad columns."""
    out = np.full((128, JP), pad_val, np.float16)
    out[:, :FREE] = flat.astype(np.float16)
    return out


def _build_nc():
    import concourse.bass as bass
    import concourse.bacc as bacc
    import concourse.mybir as mybir
    from concourse.tile import TileContext

    f32, f16, i32 = mybir.dt.float32, mybir.dt.float16, mybir.dt.int32
    AF = mybir.ActivationFunctionType
    OP = mybir.AluOpType
    ET = mybir.EngineType

    nc = bacc.Bacc("TRN2", target_bir_lowering=False, debug=False)
    jad = nc.dram_tensor("jad", [128, 2 * JP], f16, kind="ExternalInput")
    meta = nc.dram_tensor("meta", [128, NMETA], f32, kind="ExternalInput")
    out = nc.dram_tensor("out", [128, NCOLS], f32, kind="ExternalOutput")

    # patterns: [partition][(pair)][h][flat (w,d) row]
    JPP = 2 * JP                  # fused [J | A] row length (13.4KB rows)
    P1 = [[JPP, 128], [SW * SD, BH], [1, ROW]]                # single B-view
    P2 = [[JPP, 128], [2, 2], [SW * SD, BH], [1, ROW]]        # dd=+-1 pair
    PM = [[JPP, 128], [1, MROW]]                              # mini B slice
    # central views live in the compact on-chip Jc/Ac tiles [128, NQ]
    C1 = [[NQ, 128], [ROW, BH], [1, ROW]]                     # central
    C2B = [[NQ, 128], [0, 2], [ROW, BH], [1, ROW]]            # bcast pair
    CM = [[NQ, 128], [1, MROW]]                               # mini/G slice

    with TileContext(nc) as tc:
        with tc.tile_pool(name="pers", bufs=1) as pers, \
             tc.tile_pool(name="dp2", bufs=3) as dp2, \
             tc.tile_pool(name="qp2", bufs=3) as qp2, \
             tc.tile_pool(name="ep2", bufs=3) as ep2, \
             tc.tile_pool(name="vp2", bufs=3) as vp2, \
             tc.tile_pool(name="dp1", bufs=3) as dp1, \
             tc.tile_pool(name="qp1", bufs=3) as qp1, \
             tc.tile_pool(name="ep1", bufs=3) as ep1, \
             tc.tile_pool(name="vp1", bufs=3) as vp1, \
             tc.tile_pool(name="gp", bufs=1) as gp:
            JA = pers.tile([128, JPP], f16, tag="JA")
            Jc = pers.tile([128, NQ], f16, tag="Jc")
            metatile = pers.tile([128, NMETA], f32, tag="meta")
            acc = pers.tile([128, NCOLS], f32, tag="acc")
            Q1 = pers.tile([128, NQ], f16, tag="Q1")
            Q2 = pers.tile([128, 2 * NQ], f16, tag="Q2")
            fin = pers.tile([128, NQ], f16, tag="fin")

            nc.vector.memset(acc[:], 0.0)
            # meta first on the SP ring (tiny; the register loads gate all
            # compute), then J on SP; A rides the ACT ring in parallel.
            nc.sync.dma_start(metatile[:], meta[:])
            nc.sync.dma_start(JA[:], jad[:])

            t3v = metatile[:, 0:MROW]
            BIAS0 = MROW
            biasv = metatile[:, BIAS0:BIAS0 + 7]   # pair0..2, sing0..2, mini
            INT0 = BIAS0 + 7
            # ints: pair joffs 0..2 | single joffs 3..5 | mini jB 6 |
            #       mini cA 7 | gjoff 8
            _, dvv = nc.values_load_multi_w_load_instructions(
                metatile[0:1, INT0:INT0 + 16].bitcast(i32),
                engines=(ET.DVE,), min_val=0,
                max_val=JP + CENT + 1 + OFFB,
                skip_runtime_bounds_check=True)
            pv, sv, mv = dvv[0:3], dvv[3:6], dvv[6:8]
            pva, sva, mba, mca = dvv[8:11], dvv[11:14], dvv[14], dvv[15]
            gval = nc.values_load(
                metatile[0:1, INT0 + 16:INT0 + 17].bitcast(i32),
                engines=(ET.Activation,), min_val=0, max_val=NQ - MROW,
                skip_runtime_bounds_check=True)

            # compact central copies (strided -> contiguous, on DVE)
            Jsrc = bass.AP(JA.tensor, CENT, P1)
            nc.vector.tensor_copy(
                Jc[:].rearrange("p (a b) -> p a b", a=BH, b=ROW), Jsrc)

            J_A1 = bass.AP(Jc.tensor, 0, C1)
            J_A2 = bass.AP(Jc.tensor, 0, C2B)
            a_A1 = bass.AP(JA.tensor, JP + CENT, P1)

            # ---- mini + G first: they only need J/meta, fill the DMA
            # shadow on both engines (v/ms parts emitted later) ----
            md = gp.tile([128, MROW], f16, tag="md")
            nc.vector.tensor_tensor(
                md[:], bass.AP(JA.tensor, mv[0], PM),
                bass.AP(Jc.tensor, mv[1], CM), OP.subtract)
            mq = gp.tile([128, MROW], f16, tag="mq")
            nc.vector.tensor_tensor(mq[:], md[:], md[:], OP.mult)
            me = gp.tile([128, MROW], f16, tag="me")
            nc.scalar.activation(me[:], mq[:], AF.Exp, scale=-S2,
                                 bias=biasv[:, 6:7],
                                 accum_out=acc[:, 6:7])
            qg = gp.tile([128, MROW], f16, tag="qg")
            nc.scalar.activation(
                qg[:], bass.AP(Jc.tensor, gval, CM), AF.Square, scale=SQ)
            ag = gp.tile([128, MROW], f32, tag="ag")
            nc.vector.scalar_tensor_tensor(
                ag[:], qg[:], -1.0, t3v[:, 0:MROW], OP.mult, OP.add)
            eg = gp.tile([128, MROW], f16, tag="eg")
            nc.scalar.activation(eg[:], ag[:], AF.Exp,
                                 accum_out=acc[:, 10:11])

            def pair_slot(j, first):
                dt = dp2.tile([128, 2, BH, ROW], f16, tag="d2")
                nc.vector.tensor_tensor(
                    dt[:], bass.AP(JA.tensor, pv[j], P2), J_A2, OP.subtract)
                qt = qp2.tile([128, 2, BH, ROW], f16, tag="q2")
                nc.scalar.activation(qt[:], dt[:], AF.Square, scale=SQ)
                et = ep2.tile([128, 2, BH, ROW], f16, tag="e2")
                nc.scalar.activation(et[:], qt[:], AF.Exp, scale=-1.0,
                                     bias=biasv[:, j:j + 1],
                                     accum_out=acc[:, j:j + 1])
                if first:
                    nc.vector.tensor_tensor(
                        Q2[:].rearrange("p (a b c) -> p a b c", a=2, b=BH),
                        et[:], bass.AP(JA.tensor, pva[j], P2), OP.mult)
                else:
                    vt = vp2.tile([128, 2, BH, ROW], f16, tag="v2")
                    nc.vector.tensor_tensor(
                        vt[:], et[:], bass.AP(JA.tensor, pva[j], P2), OP.mult)
                    nc.vector.tensor_tensor(
                        Q2[:], Q2[:],
                        vt[:].rearrange("p a b c -> p (a b c)"), OP.add)

            def single_slot(j, mode):
                # mode: "init" -> v writes Q1; "add" -> Q1 += v;
                #       "stt" -> bypass Q1, STT-accum into its own column
                dt = dp1.tile([128, BH, ROW], f16, tag="d1")
                nc.vector.tensor_tensor(
                    dt[:], bass.AP(JA.tensor, sv[j], P1), J_A1, OP.subtract)
                qt = qp1.tile([128, BH, ROW], f16, tag="q1")
                nc.scalar.activation(qt[:], dt[:], AF.Square, scale=SQ)
                escale = -1.0
                et = ep1.tile([128, BH, ROW], f16, tag="e1")
                nc.scalar.activation(et[:], qt[:], AF.Exp, scale=escale,
                                     bias=biasv[:, NPAIR + j:NPAIR + j + 1],
                                     accum_out=acc[:, NPAIR + j:NPAIR + j + 1])
                if mode == "init":
                    nc.vector.tensor_tensor(
                        Q1[:].rearrange("p (a b) -> p a b", a=BH),
                        et[:], bass.AP(JA.tensor, sva[j], P1), OP.mult)
                    return
                vt = vp1.tile([128, BH, ROW], f16, tag="v1")
                nc.vector.tensor_tensor(
                    vt[:], et[:], bass.AP(JA.tensor, sva[j], P1), OP.mult)
                if mode == "add":
                    nc.vector.tensor_tensor(
                        Q1[:], Q1[:],
                        vt[:].rearrange("p a b -> p (a b)"), OP.add)
                else:
                    st = gp.tile([128, BH, ROW], f16, tag="st")
                    nc.vector.scalar_tensor_tensor(
                        st[:], vt[:], 1.0, a_A1, OP.mult, OP.mult,
                        accum_out=acc[:, 11:12])

            single_slot(0, "init")
            pair_slot(0, True)
            # mini product part (needs A)
            mvt = gp.tile([128, MROW], f16, tag="mv")
            nc.vector.tensor_tensor(
                mvt[:], me[:], bass.AP(JA.tensor, mba, PM), OP.mult)
            ms = gp.tile([128, MROW], f16, tag="ms")
            nc.vector.scalar_tensor_tensor(
                ms[:], mvt[:], 1.0, bass.AP(JA.tensor, mca, PM),
                OP.mult, OP.mult, accum_out=acc[:, 9:10])

            pair_slot(1, False)
            single_slot(1, "add")
            pair_slot(2, False)

            # single2 front half (sub/Square/Exp) - independent of Q1/Q2
            s2d = dp1.tile([128, BH, ROW], f16, tag="d1")
            nc.vector.tensor_tensor(
                s2d[:], bass.AP(JA.tensor, sv[2], P1), J_A1, OP.subtract)
            s2q = qp1.tile([128, BH, ROW], f16, tag="q1")
            nc.scalar.activation(s2q[:], s2d[:], AF.Square, scale=SQ)
            s2e = ep1.tile([128, BH, ROW], f16, tag="e1")
            nc.scalar.activation(s2e[:], s2q[:], AF.Exp, scale=-1.0,
                                 bias=biasv[:, NPAIR + 2:NPAIR + 3],
                                 accum_out=acc[:, NPAIR + 2:NPAIR + 3])

            # collapse pair lanes, fold into Q1, and run the final
            # reduction BEFORE single2's product ops so merge+fin overlap
            # single2's Square/Exp on ACT (single2 bypasses Q1 via its
            # own STT column, so the order is sound)
            nc.vector.tensor_tensor(Q2[:, 0:NQ], Q2[:, 0:NQ],
                                    Q2[:, NQ:2 * NQ], OP.add)
            nc.vector.tensor_tensor(Q1[:], Q1[:], Q2[:, 0:NQ], OP.add)
            # ---- final: col8 = sum aA * Q1 ----
            nc.vector.scalar_tensor_tensor(
                fin[:].rearrange("p (a b) -> p a b", a=BH, b=ROW),
                Q1[:].rearrange("p (a b) -> p a b", a=BH, b=ROW),
                1.0, a_A1, OP.mult, OP.mult, accum_out=acc[:, 8:9])

            # single2 back half: v-mult + STT into its own column
            s2v = vp1.tile([128, BH, ROW], f16, tag="v1")
            nc.vector.tensor_tensor(
                s2v[:], s2e[:], bass.AP(JA.tensor, sva[2], P1), OP.mult)
            s2s = gp.tile([128, BH, ROW], f16, tag="st")
            nc.vector.scalar_tensor_tensor(
                s2s[:], s2v[:], 1.0, a_A1, OP.mult, OP.mult,
                accum_out=acc[:, 11:12])

            nc.sync.dma_start(out[:], acc[:])
    nc.compile()
    return nc


def _host_tables(sample, spacing):
    """Per-core meta arrays."""
    sp = np.asarray(spacing, dtype=np.float64)[:, 0]
    pairs, singles = _pair_single_slots()

    # t3 = ln(noob) - 0.5*msq_center (NEG where noob == 0), central packing
    h = np.arange(H)[:, None, None]
    w = np.arange(W)[None, :, None]
    d = np.arange(D)[None, None, :]
    msq_c = ((sp[0] * h) ** 2 + (sp[1] * w) ** 2 + (sp[2] * d) ** 2) / SXY ** 2
    cnt = ((np.minimum(h, RH) + np.minimum(H - 1 - h, RH) + 1)
           * (np.minimum(w, RW) + np.minimum(W - 1 - w, RW) + 1)
           * (np.minimum(d, RD) + np.minimum(D - 1 - d, RD) + 1))
    noob = (2 * RH + 1) * (2 * RW + 1) * (2 * RD + 1) - cnt
    t3full = np.where(noob > 0, np.log(np.maximum(noob, 1)) - 0.5 * msq_c, NEG)
    t3b = np.empty((128, BH, BW, D), np.float32)
    for hb in range(NHB):
        for wb in range(NWB):
            t3b[hb * NWB + wb] = t3full[hb * BH:(hb + 1) * BH,
                                        wb * BW:(wb + 1) * BW, :]
    t3flat = t3b.reshape(128, BH * BW * D)

    def bias_of(dh, dw, dd):
        msq = ((sp[0] * dh) ** 2 + (sp[1] * dw) ** 2
               + (sp[2] * dd) ** 2) / SXY ** 2
        return -0.5 * msq

    metas = []
    for c in range(NCORES):
        m = np.zeros((128, NMETA), np.float32)
        # G/mini slice: h-row c//2, col half c%2 of the 272-wide flat row
        sl = (c // 2) * SW * SD + (c % 2) * MROW
        # map on-chip slice positions (d incl halo) to central t3 values
        t3s = np.full((128, MROW), NEG, np.float32)
        r0 = c // 2
        base = (c % 2) * MROW
        for i in range(MROW):
            wcol, dcol = divmod(base + i, SD)
            if 1 <= dcol <= D:
                t3s[:, i] = t3flat[:, (r0 * BW + wcol) * D + (dcol - 1)]
        m[:, 0:MROW] = t3s

        B0 = MROW
        joff_p = np.zeros(3, np.int32)
        joff_s = np.zeros(3, np.int32)
        for j in range(NPAIR):
            dh, dw = pairs[3 * c + j]
            j0 = dh * SW * SD + (RW + dw) * SD + (RD - 1)   # dd=-1 lane
            assert j0 % 2 == 0
            joff_p[j] = j0                                  # copy-0 coords
            m[:, B0 + j] = bias_of(dh, dw, 1)
        for j in range(NSING):
            dh, dw = singles[3 * c + j]
            j0 = dh * SW * SD + (RW + dw) * SD + RD         # dd=0
            assert j0 % 2 == 1
            joff_s[j] = j0                                  # copy-0, odd
            m[:, B0 + NPAIR + j] = bias_of(dh, dw, 0)
        m[:, B0 + 6] = bias_of(0, 0, 1)                     # mini
        I0 = B0 + 7
        m[0, I0:I0 + 3] = joff_p.view(np.float32)
        m[0, I0 + 3:I0 + 6] = joff_s.view(np.float32)
        # mini: B = central+1 (dd=+1) in stored J coords; A-side and the
        # G input are slices of the compact central Jc/Ac tiles; the
        # second block of offsets addresses the A half (+JP) of the
        # fused [J | A] tile
        mb = CENT + 1 + sl
        csl = (c // 2) * ROW + (c % 2) * MROW
        m[0, I0 + 6] = np.int32(mb).view(np.float32)
        m[0, I0 + 7] = np.int32(csl).view(np.float32)
        m[0, I0 + 8:I0 + 11] = (joff_p + JP).view(np.float32)
        m[0, I0 + 11:I0 + 14] = (joff_s + JP).view(np.float32)
        m[0, I0 + 14] = np.int32(mb + JP).view(np.float32)
        m[0, I0 + 15] = np.int32(JP + CENT + sl).view(np.float32)
        m[0, I0 + 16] = np.int32(csl).view(np.float32)      # gjoff
        metas.append(m)
    return metas


def _host_inputs(y_hat_softmax, sample, spacing):
    y0 = np.asarray(y_hat_softmax, dtype=np.float32)[0, 0]
    I = np.asarray(sample, dtype=np.float32)[0, 0]
    jd = _single_f16(_pack(I, BIG), 0.0)
    ad = _single_f16(_pack(1.0 - 2.0 * y0, 1.0), 1.0)
    jad = np.concatenate([jd, ad], axis=1)
    metas = _host_tables(sample, spacing)
    return [{"jad": jad, "meta": metas[c]} for c in range(NCORES)]


def kernel(y_hat_softmax, sample, spacing):
    from concourse.bass_utils import run_bass_kernel_spmd

    in_maps = _host_inputs(y_hat_softmax, sample, spacing)
    nc = _build_nc()
    res = run_bass_kernel_spmd(nc, in_maps, core_ids=list(range(NCORES)))
    total = 0.0
    for r in res.results:
        o = r["out"].astype(np.float64)
        # cols 0..6 = sum E per slot (pairs, singles, mini); col8 = final
        # sum aA*Q1; col9 = mini product sum; col10 = G; col11 = single2
        # product sum
        total += (o[:, 0:7].sum() - o[:, 8].sum() - o[:, 9].sum()
                  + o[:, 10].sum() - o[:, 11].sum())
    return np.array(total / DENOM, dtype=np.float32)


if __name__ == "__main__":
    rng = np.random.default_rng(0)
    logits = rng.standard_normal((1, 2, H, W, D)).astype(np.float32)
    e = np.exp(logits - logits.max(axis=1, keepdims=True))
    yh = (e / e.sum(axis=1, keepdims=True)).astype(np.float32)
    smp = rng.standard_normal((1, 1, H, W, D)).astype(np.float32)
    spc = rng.uniform(0.5, 2.0, (3, 1)).astype(np.float32)
    print(kernel(yh, smp, spc))
